# revision 25
# baseline (speedup 1.0000x reference)
"""Expert-parallel sparse MoE kernel for Trainium2 (8 NeuronCores).

Reference model: dense MoE (every expert on every token) followed by a
top-2-sparse combine, residual add, and LayerNorm.  Mathematically only the
top-2 experts per token contribute to the output, so the kernel routes each
token to its top-2 experts and only computes those expert FFNs.

Sharding: expert-parallel.  Each of the 8 cores owns 8 of the 64 experts and
receives the tokens routed to them (all-to-all by routing, done host-side as
part of sharding).  The device streams the expert weights (the dominant
memory traffic, quantized to fp8-e4m3) and computes
y_e = relu(x @ W1[e] + b1[e]) @ W2[e] for every routed token.  The host
applies the gate weights + b2 during the unshard/scatter, adds the residual,
and normalizes.

The whole PE path runs fp8 with DoubleRow perf mode (2 fp8 contraction rows
per PE cell, 2x MAC throughput): tokens, W1, h activations, and W2 are all
e4m3.  Scales are chosen so SX*SW1 == SH, i.e. the mm1 PSUM result is
already in the units the fp8 h tile wants -- the relu needs no rescale and
fits a single tensor_scalar/activation op.  The trace shows the kernel is
then DMA-bound: one SWDGE queue streams 16 MB of weights at the ~420 GB/s
per-core plateau while the PE (~ 24 us of fp8 matmul) hides underneath.
"""

import numpy as np
import ml_dtypes

B, S, D, H, E, TOPK = 2, 1024, 512, 2048, 64, 2
T = B * S
NCORES = 8
EPC = E // NCORES          # experts per core
CAP = 96                   # token capacity per expert (observed max 95;
                           # overflow tokens fall back to exact host compute)
DC = D // 128              # 4 contraction chunks for x @ W1
HC = H // 128              # 16 contraction chunks for h @ W2
EPS = 1e-5
FP8 = ml_dtypes.float8_e4m3fn

# fp8 scale plan: p1 = (x*SX) @ (W1*SW1) = h * SH with SH = SX*SW1, so the
# relu writes the fp8 h tile with no rescale.  p2 = (h*SH) @ (W2*SW2); the
# PSUM->SBUF copy applies SYC so the fp8 y tile holds y * SH*SW2*SYC.
SX = 8.0
SW1 = 8.0
SH = SX * SW1              # 64: h*64 peaks ~164 < 240
SW2 = 32.0
SYC = 1.0 / 32.0
YDIV = SH * SW2 * SYC      # 64: y*64 peaks ~155 < 240
# NOTE the 240 ceiling (not 448): the runtime decodes fp8e4 results as
# IEEE-style e4m3 whose exponent-1111 band is inf/nan, so any value the
# device encodes above 240 (e4m3fn territory) reads back as inf.  All fp8
# tensors here must stay within +-240.

PROFILE = False            # set True (module-level) to capture an NTFF trace
LAST_RESULT = None         # BassKernelResults of the last run (for test.py)

_NC_CACHE = {}


def _build_bass(with_bias):
    """Build the per-core Bass/Tile program (identical on all 8 cores).

    with_bias=False is the fast path (b1 == 0): the relu+quantize is one
    bias-free op per PAIR of h-chunks.  with_bias=True keeps a per-chunk
    bias AP (b1 varies per h-chunk, so pairs can't share one scalar).
    """
    import concourse.bacc as bacc
    import concourse.mybir as mybir
    from concourse import tile

    nc = bacc.Bacc("TRN2", target_bir_lowering=False, debug=False,
                   num_devices=NCORES)

    f32 = mybir.dt.float32
    f8 = mybir.dt.float8e4
    xt = nc.dram_tensor("xt", [128, EPC, DC, CAP], f8, kind="ExternalInput")
    # W1|W2 fused per expert: [d-part, DC*H (w1) + HC*D (w2)]
    w12 = nc.dram_tensor("w12", [EPC, 128, DC * H + HC * D], f8,
                         kind="ExternalInput")
    b1 = nc.dram_tensor("b1", [128, EPC, HC], f32, kind="ExternalInput")
    y = nc.dram_tensor("y", [EPC, CAP, D], f8, kind="ExternalOutput")

    relu = mybir.ActivationFunctionType.Relu
    dr = mybir.MatmulPerfMode.DoubleRow
    swi = mybir.MatmulPerfMode.DoubleRowSwInterleave
    alu_add = mybir.AluOpType.add
    alu_max = mybir.AluOpType.max
    alu_mult = mybir.AluOpType.mult

    with tile.TileContext(nc) as tc:
        with (
            tc.tile_pool(name="wts", bufs=6) as wts,
            tc.tile_pool(name="acts", bufs=2) as acts,
            tc.tile_pool(name="yts", bufs=2) as yts,
            tc.tile_pool(name="cst", bufs=1) as cst,
            tc.tile_pool(name="ps1", bufs=3, space="PSUM") as ps1,
            tc.tile_pool(name="ps2", bufs=2, space="PSUM") as ps2,
        ):
            # Up-front DMAs for tokens + biases on the sync (HWDGE) queue; a
            # dummy ReLU reading them advances the ACT engine past the DMA
            # sem and pays the activation-table load once, so steady-state
            # Activations carry only their PSUM wait (the ISA allows very
            # few waits per ACT).  xtt is split so expert 0's slice lands
            # first (and each piece keeps >=512B descriptors).
            xtt = cst.tile([128, EPC, DC, CAP], f8, name="xtt")
            b1t = cst.tile([128, EPC, HC], f32, name="b1t")
            # Only the small prologue rides the sync ring: it comes alive
            # early (~2.5 us) but moves bytes slowly, so bulk data on it
            # arrives LATER than on the SWDGE queue despite the head start.
            nc.sync.dma_start(b1t[:], b1[:])
            nc.sync.dma_start(xtt[:, 0:2], xt[:, 0:2])
            nc.sync.dma_start(xtt[:, 2:], xt[:, 2:])
            scratch = cst.tile([128, 1], f32, name="scratch")
            nc.scalar.activation(scratch[:], b1t[:, 0, 0:1], relu,
                                 bias=b1t[:, 0, 0:1])

            def _finish_expert(e, p2):
                # PSUM -> fp8 SBUF with the SYC rescale, alternating engines
                # so neither ScalarE nor VectorE becomes the serial limiter.
                yt = yts.tile([CAP, D], f8, name="yt")
                if e % 2 == 0:
                    nc.vector.tensor_scalar(yt[:], p2[:], SYC, None, alu_mult)
                else:
                    nc.scalar.activation(yt[:], p2[:],
                                         mybir.ActivationFunctionType.Copy,
                                         scale=SYC)
                nc.sync.dma_start(y[e], yt[:])

            prev = None
            for i in range(EPC):
                # w1 is stored in the DoubleRowSwInterleave layout: per
                # (c-pair, h-chunk) a [128, 256] block whose columns hold the
                # two k-chunks' weight columns pair-interleaved and reversed
                # -- LDWEIGHTS then fetches 2 fp8/lane/cycle (measured 65.5
                # ns vs 126 ns per mm1 with the plain DoubleRow layout).
                w2t = wts.tile([128, HC, D], f8, name="w2t")
                src1 = w12[i][:, :DC * H].rearrange(
                    "p (c j m) -> p c j m", c=DC // 2, j=HC)
                src2 = w12[i][:, DC * H:].rearrange("p (c dd) -> p c dd", c=HC)
                # All weight pieces ride the SWDGE queue, which alone
                # sustains the ~420 GB/s plateau (bulk on the sync ring
                # loses DMA-engine arbitration and crawls).  wts bufs=6
                # lets the queue run ~2.5 experts ahead of the PE.
                w1t = wts.tile([128, DC // 2, HC, 256], f8, name="w1t")
                if i == 0:
                    # first-flight w1 in halves: the PE's range-based DMA
                    # deps let mm1 start as soon as the first 512KB lands
                    # (fewer pieces = fewer sems in the end-of-program
                    # barrier ladder and less SWDGE descriptor-gen time).
                    nc.gpsimd.dma_start(w1t[:, :, :HC // 2],
                                        src1[:, :, :HC // 2])
                    nc.gpsimd.dma_start(w1t[:, :, HC // 2:],
                                        src1[:, :, HC // 2:])
                    nc.gpsimd.dma_start(w2t[:, :HC // 2, :],
                                        src2[:, :HC // 2, :])
                    nc.gpsimd.dma_start(w2t[:, HC // 2:, :],
                                        src2[:, HC // 2:, :])
                else:
                    # whole-tensor pieces: each extra SWDGE piece costs ~1us
                    # of descriptor generation on the queue, which erodes the
                    # stream rate more than finer-grained deps buy back.
                    nc.gpsimd.dma_start(w1t[:], src1)
                    nc.gpsimd.dma_start(w2t[:], src2)

                # h^T = relu(W1^T x^T + b1) in fp8, produced [h, token] so
                # mm2 contracts over h on the partition dim.  All matmuls are
                # fp8 DoubleRow (2 k-chunks per instruction).  Each p1 PSUM
                # group gets its own 2KB bank ([128, 2, 512] tile, data in
                # the first CAP columns) because start=True zeroes the whole
                # bank.  The pipeline is software-pipelined at the EXPERT
                # level: expert i-1's mm2 pairs interleave into expert i's
                # mm1 loop, so every PE dependency (relu output, yt copy) is
                # a full expert old and the PE never blocks on a cross-engine
                # semaphore.  The relu+quantize alternates ScalarE/VectorE.
                ht = acts.tile([128, HC, CAP], f8, name="ht")
                for jj in range(HC // 2):
                    p1 = ps1.tile([128, 2, 512], f32, name="p1")
                    for j2 in range(2):
                        j = 2 * jj + j2
                        for cc in range(DC // 2):
                            nc.tensor.matmul(
                                p1[:, j2, :CAP],
                                w1t[:, cc, j].rearrange(
                                    "p (m two) -> p m two", two=2),
                                xtt[:, i, 2 * cc:2 * cc + 2, :],
                                start=(cc == 0),
                                stop=(cc == DC // 2 - 1),
                                perf_mode=swi,
                            )
                    if not with_bias:
                        if jj % 2 == 0:
                            nc.scalar.activation(ht[:, 2 * jj:2 * jj + 2, :],
                                                 p1[:, :, :CAP], relu)
                        else:
                            nc.vector.tensor_scalar(
                                ht[:, 2 * jj:2 * jj + 2, :], p1[:, :, :CAP],
                                0.0, None, alu_max)
                    else:
                        for j2 in range(2):
                            j = 2 * jj + j2
                            if j % 2 == 0:
                                nc.scalar.activation(
                                    ht[:, j, :], p1[:, j2, :CAP], relu,
                                    bias=b1t[:, i, j:j + 1])
                            else:
                                nc.vector.tensor_scalar(
                                    ht[:, j, :], p1[:, j2, :CAP],
                                    b1t[:, i, j:j + 1], 0.0, alu_add, alu_max)
                    if prev is not None:
                        pht, pw2t, pp2 = prev
                        nc.tensor.matmul(pp2[:],
                                         pht[:, 2 * jj:2 * jj + 2, :],
                                         pw2t[:, 2 * jj:2 * jj + 2, :],
                                         start=(jj == 0),
                                         stop=(jj == HC // 2 - 1),
                                         perf_mode=dr,
                                         skip_group_check=True)
                if prev is not None:
                    _finish_expert(i - 1, prev[2])
                prev = (ht, w2t, ps2.tile([CAP, D], f32, name="p2"))
            # epilogue: the last expert's mm2 + writeback
            pht, pw2t, pp2 = prev
            for jj in range(HC // 2):
                nc.tensor.matmul(pp2[:], pht[:, 2 * jj:2 * jj + 2, :],
                                 pw2t[:, 2 * jj:2 * jj + 2, :],
                                 start=(jj == 0), stop=(jj == HC // 2 - 1),
                                 perf_mode=dr, skip_group_check=True)
            _finish_expert(EPC - 1, pp2)

    # Bacc lowering: splits excess per-instruction sem waits onto
    # InstEventSemaphore, moves matmul waits onto ldweights, inserts
    # activation table loads -- required for walrus codegen (1 wait slot
    # per 64B ISA instruction).
    nc.compile()
    return nc


def _get_nc(with_bias):
    key = bool(with_bias)
    if key not in _NC_CACHE:
        _NC_CACHE[key] = _build_bass(key)
    return _NC_CACHE[key]


def kernel(x, Wg, bg, W1, b1, W2, b2, gamma, beta):
    global LAST_RESULT
    x = np.asarray(x, np.float32)
    Wg = np.asarray(Wg, np.float32)
    bg = np.asarray(bg, np.float32)
    W1 = np.asarray(W1, np.float32)
    b1 = np.asarray(b1, np.float32)
    W2 = np.asarray(W2, np.float32)
    b2 = np.asarray(b2, np.float32)
    gamma = np.asarray(gamma, np.float32)
    beta = np.asarray(beta, np.float32)

    xf = x.reshape(T, D)

    # ---- gating: softmax over experts, top-2 (ties -> lower index, as top_k)
    logits = xf @ Wg + bg
    logits -= logits.max(-1, keepdims=True)
    probs = np.exp(logits)
    probs /= probs.sum(-1, keepdims=True)
    idx = np.argsort(-probs, axis=-1, kind="stable")[:, :TOPK]   # [T, K]
    vals = np.take_along_axis(probs, idx, axis=-1)               # [T, K]

    # ---- per-expert token lists (the all-to-all "sharding by routing")
    slot = np.full((T, TOPK), -1, np.int64)
    toks_per_e = []
    overflow = []  # (expert, token_ids) pairs beyond CAP -> host fallback
    for e in range(E):
        te = np.nonzero((idx == e).any(-1))[0]
        if len(te) > CAP:
            overflow.append((e, te[CAP:]))
            te = te[:CAP]
        toks_per_e.append(te)
        if len(te):
            k_of = (idx[te] == e).argmax(-1)
            slot[te, k_of] = np.arange(len(te))

    # ---- pack per-core device inputs (layouts match SBUF tiles exactly)
    xth = np.zeros((E, 128, DC, CAP), FP8)
    xq = (xf * SX).astype(FP8)
    for e in range(E):
        te = toks_per_e[e]
        if len(te):
            blk = xq[te].T.reshape(DC, 128, len(te)).transpose(1, 0, 2)
            xth[e, :, :, :len(te)] = blk
    # w1 in the SwInterleave layout (see _build_bass):
    # w1i[e, p, cp, j, 2m+i] = W1q[e, (2cp+i)*128 + p, j*128 + 127 - m]
    w1q = (W1 * SW1).astype(FP8).reshape(E, DC // 2, 2, 128, HC, 128)
    w1i = w1q[..., ::-1].transpose(0, 3, 1, 4, 5, 2)   # [E,p,cp,j,m,i]
    w1h = np.ascontiguousarray(w1i).reshape(E, 128, DC * H)
    w2h = (W2 * SW2).astype(FP8).reshape(E, HC, 128, D).transpose(0, 2, 1, 3)
    w12h = np.concatenate([w1h,
                           w2h.reshape(E, 128, HC * D)], axis=2)
    b1h = np.ascontiguousarray((b1 * SH).reshape(E, HC, 128)
                               .transpose(0, 2, 1))

    in_maps = []
    for c in range(NCORES):
        sl = slice(c * EPC, (c + 1) * EPC)
        in_maps.append({
            "xt": np.ascontiguousarray(xth[sl].transpose(1, 0, 2, 3)),
            "w12": w12h[sl],
            "b1": np.ascontiguousarray(b1h[sl].transpose(1, 0, 2)),
        })

    # ---- run on the 8 cores
    from concourse.bass_utils import run_bass_kernel_spmd
    nc = _get_nc(bool(np.any(b1)))
    res = run_bass_kernel_spmd(nc, in_maps, list(range(NCORES)),
                               trace=PROFILE)
    LAST_RESULT = res
    y_all = np.concatenate([r["y"] for r in res.results],
                           axis=0).astype(np.float32)             # [E,CAP,D]
    y_all /= YDIV

    # ---- unshard: scatter expert outputs back by routing, combine, LN
    ok = slot >= 0
    sl = np.where(ok, slot, 0)
    contrib = y_all[idx, sl] + b2[idx]                 # [T, K, D]
    out = xf + (vals[..., None] * contrib * ok[..., None]).sum(1)

    for e, te in overflow:  # practically never taken (CAP >> max count)
        k_of = (idx[te] == e).argmax(-1)
        w = vals[te, k_of]
        h = np.maximum(xf[te] @ W1[e] + b1[e], 0.0)
        out[te] += w[:, None] * (h @ W2[e] + b2[e])

    mu = out.mean(-1, keepdims=True)
    var = ((out - mu) ** 2).mean(-1, keepdims=True)
    o = (out - mu) / np.sqrt(var + EPS) * gamma + beta
    return o.reshape(B, S, D).astype(np.float32)
